# revision 11
# baseline (speedup 1.0000x reference)
"""2-layer GAT (PyG GATConv, concat=False, self-loops) on 8 Trainium2 cores.

Design (v2): nodes/edges partitioned by destination across 8 cores; each
core owns a 6250-dst range, dsts degree-sorted into 49 blocks of 128 PSUM
lanes.  Per-edge source features are fetched with dma_gather from a
256B-per-node table in HBM (pure bf16 h, h-major); per-edge attention
logits are recomputed on-chip with a DVE dot against a replicated a_src
row, so rows stay at the 256B dma_gather minimum.  The node table lives in
g-space (8 slabs of 6251 rows, one junk row per slab); one int16 index
stream serves both layers.  int16 gather indices cover 32767 rows, so
edges are split between two overlapping views (lo=[0,32767),
hi=[17241,50008)) with per-lane balancing of the 31% of sources that both
views can address.  Self-loop edges are excluded from the gather: each
core streams its own dst rows sequentially (also providing a_dst) and adds
the self contribution in the epilogue.  Padding slots point at a sentinel
row v with a_src@v = -300 so exp(LRelu(...)) ~ 0.  Scatter is free: slots
accumulate into PSUM via identity-weight matmuls (3 columns per matmul).
Layer boundary: AllGather of the bf16-transposed layer-1 output, then a
replicated stage-A rebuilds the layer-2 table on every core.
"""
import sys
sys.path.insert(0, "/opt/trn_rl_repo")

import numpy as np
import ml_dtypes

import concourse.bass as bass
import concourse.bacc as bacc
import concourse.mybir as mybir
from concourse.bass_utils import run_bass_kernel_spmd
from concourse.tile import TileContext

N = 50000
E = 1600000
IN = 128
H = 4
F = 32
NEG = 0.2
NCORES = 8
PERC = N // NCORES           # 6250
SLAB = PERC + 1              # 6251 (junk row at pos 6250)
TROWS = NCORES * SLAB        # 50008
NBLK = (PERC + 127) // 128   # 49
LO_END = 32767               # lo view = rows [0, 32767)
HI_START = TROWS - 32767     # 17241; hi view = rows [17241, 50008)
SENT_ROW = 3 * SLAB + PERC   # 25003 (core 3's junk row; inside the overlap)
RW = 64                      # table row: 64 f32 slots = 256B (128 bf16 h)
MAXC = int(__import__('os').environ.get('GAT_MAXC', '30'))  # max gather columns per dma_gather call
GRP = 3                      # aggregation matmul: 3 slot-columns per matmul


def _chunks(n):
    return [min(MAXC, n - s) for s in range(0, n, MAXC)]


def _pack_idx(idx_flat):
    """[n] -> [128, n/16] int16; idx i -> (partition i%16, col i//16), x8."""
    n = idx_flat.shape[0]
    assert n % 16 == 0
    a = idx_flat.reshape(n // 16, 16).T.astype(np.int16)
    return np.ascontiguousarray(np.tile(a, (8, 1)))


def _preprocess(edge_index):
    """Index preprocessing. Returns per-core gidx + shared structure."""
    src0 = edge_index[0].astype(np.int64)
    dst0 = edge_index[1].astype(np.int64)
    deg = np.bincount(dst0, minlength=N)          # gather-degree (no self)

    def build_perms(lo_cnt):
        perms = []
        pos_of = np.empty(N, np.int64)
        for c in range(NCORES):
            ids = np.arange(c * PERC, (c + 1) * PERC)
            order = np.lexsort((-lo_cnt[ids], -deg[ids]))
            perm = ids[order]
            perms.append(perm)
            pos_of[perm] = np.arange(PERC)
        return perms, pos_of

    # pass 1: degree only -> provisional rows -> per-dst must-lo counts
    perms, pos_of = build_perms(np.zeros(N, np.int64))
    for _ in range(2):
        row_of = (np.arange(N) // PERC) * SLAB + pos_of
        must_lo = row_of[src0] < HI_START
        lo_cnt = np.bincount(dst0[must_lo], minlength=N)
        perms, pos_of = build_perms(lo_cnt)

    row_of = (np.arange(N) // PERC) * SLAB + pos_of
    r_src = row_of[src0]
    kind = np.where(r_src < HI_START, 0, np.where(r_src >= LO_END, 1, 2))

    lane_all = pos_of[dst0]
    core_all = dst0 // PERC
    blk_all = lane_all // 128

    # per-block slot counts nl/nh (cross-core maxima)
    A = np.zeros(NBLK, np.int64)
    B = np.zeros(NBLK, np.int64)
    C = np.zeros(NBLK, np.int64)
    cnt_l = np.zeros((NCORES, PERC), np.int64)
    cnt_h = np.zeros((NCORES, PERC), np.int64)
    cnt_f = np.zeros((NCORES, PERC), np.int64)
    for c in range(NCORES):
        m = core_all == c
        lane = lane_all[m]
        k = kind[m]
        cnt_l[c] = np.bincount(lane[k == 0], minlength=PERC)
        cnt_h[c] = np.bincount(lane[k == 1], minlength=PERC)
        cnt_f[c] = np.bincount(lane[k == 2], minlength=PERC)
    for b in range(NBLK):
        sl = slice(b * 128, min((b + 1) * 128, PERC))
        A[b] = cnt_l[:, sl].max()
        B[b] = cnt_h[:, sl].max()
        C[b] = (cnt_l[:, sl] + cnt_h[:, sl] + cnt_f[:, sl]).max()
    nl = A.copy()
    nh = np.maximum(B, C - A)
    S = int((nl + nh).sum())
    col_lo = np.zeros(NBLK, np.int64)   # column start of block's lo run
    col_hi = np.zeros(NBLK, np.int64)
    col = 0
    for b in range(NBLK):
        col_lo[b] = col
        col += nl[b]
        col_hi[b] = col
        col += nh[b]
    assert col == S

    # per-core column fill
    gidx = []
    sent_lo, sent_hi = SENT_ROW, SENT_ROW - HI_START
    for c in range(NCORES):
        m = core_all == c
        lane = lane_all[m]
        k = kind[m].copy()
        rows = r_src[m]
        # flex -> lo for the first a_i of each lane, rest -> hi
        nh_of_lane = nh[np.arange(PERC) // 128]
        a_i = np.maximum(0, cnt_h[c] + cnt_f[c] - nh_of_lane)
        # order edges by (lane, kind-rank, row); flex edges get rank by
        # position so the first a_i go to lo
        order = np.lexsort((rows, k, lane))
        lane_s, k_s, rows_s = lane[order], k[order], rows[order]
        # cumcount within (lane, kind)
        start = np.r_[True, (lane_s[1:] != lane_s[:-1]) | (k_s[1:] != k_s[:-1])]
        grp_id = np.cumsum(start) - 1
        first_pos = np.full(grp_id[-1] + 1, 1 << 62, np.int64)
        np.minimum.at(first_pos, grp_id, np.arange(len(grp_id)))
        cum = np.arange(len(grp_id)) - first_pos[grp_id]
        # stream + slot per edge
        is_lo = (k_s == 0) | ((k_s == 2) & (cum < a_i[lane_s]))
        slot = np.where(
            k_s == 0, cum,                                   # must-lo
            np.where(k_s == 1, cum,                          # must-hi
                     np.where(is_lo, cnt_l[c][lane_s] + cum,  # flex->lo
                              cnt_h[c][lane_s] + cum - a_i[lane_s])))  # flex->hi
        b_s = lane_s // 128
        colidx = np.where(is_lo, col_lo[b_s] + slot, col_hi[b_s] + slot)
        grid = np.full((S, 128), -1, np.int64)
        grid[colidx, lane_s % 128] = rows_s
        # sentinels + view-relative index
        lo_cols = np.zeros(S, bool)
        for b in range(NBLK):
            lo_cols[col_lo[b]:col_lo[b] + nl[b]] = True
        grid[lo_cols] = np.where(grid[lo_cols] < 0, sent_lo, grid[lo_cols])
        grid[~lo_cols] = np.where(grid[~lo_cols] < 0, sent_hi + HI_START,
                                  grid[~lo_cols]) - HI_START
        assert grid.min() >= 0 and grid.max() < LO_END
        gidx.append(_pack_idx(grid.reshape(-1)))

    return dict(gidx=gidx, nl=nl, nh=nh, S=S, col_lo=col_lo, col_hi=col_hi,
                perms=perms, pos_of=pos_of, row_of=row_of)


def _stage_a(nc, pools, views, w_sb, tbl, kdim):
    """h = lhsT.T @ W -> bf16 rows packed into tbl.

    views: list of (lhsT_view [kdim, W], row0); W <= 16*128. Rows go out as
    slab DMAs to tbl[row0:row0+W] (f32 [*, 64] holding 128 bf16).
    """
    sb, ps = pools
    f32, bf16 = mybir.dt.float32, mybir.dt.bfloat16
    for (view, row0) in views:
        Wt = view.shape[1]
        nt = (Wt + 127) // 128
        xsb = sb.tile([kdim, 16 * 128], bf16, tag="xa")
        nc.sync.dma_start(out=xsb[:, 0:Wt], in_=view)
        stg = sb.tile([128, 16, RW], f32, tag="sa")
        for t in range(nt):
            w = min(128, Wt - t * 128)
            psum = ps.tile([128, 128], f32, tag="pa")
            nc.tensor.matmul(out=psum[0:w, :], lhsT=xsb[:, t * 128:t * 128 + w],
                             rhs=w_sb[:], start=True, stop=True)
            nc.vector.tensor_copy(
                out=stg[0:w, t, :].bitcast(bf16),
                in_=psum[0:w, :])
        nfull = Wt // 128
        if nfull:
            nc.sync.dma_start(
                out=tbl[row0:row0 + nfull * 128, :].rearrange(
                    "(t p) c -> p t c", p=128),
                in_=stg[:, 0:nfull, :])
        if Wt % 128:
            nc.sync.dma_start(out=tbl[row0 + nfull * 128:row0 + Wt, :],
                              in_=stg[0:Wt % 128, nfull, :])


def _build_program(meta):
    nl, nh, S = meta["nl"], meta["nh"], meta["S"]
    nc = bacc.Bacc("TRN2", target_bir_lowering=False, debug=False,
                   num_devices=NCORES, dynamic_dma_scratch_size=int(__import__('os').environ.get('GAT_SCRATCH', '65536')))

    f32, bf16, i16 = mybir.dt.float32, mybir.dt.bfloat16, mybir.dt.int16
    xTg = nc.dram_tensor("xTg", [IN, TROWS], bf16, kind="ExternalInput")
    xTs = nc.dram_tensor("xTs", [IN, PERC], bf16, kind="ExternalInput")
    w1 = nc.dram_tensor("w1", [IN, H * F], bf16, kind="ExternalInput")
    w2 = nc.dram_tensor("w2", [F, H * F], bf16, kind="ExternalInput")
    asr1 = nc.dram_tensor("asr1", [128, H * F], bf16, kind="ExternalInput")
    adr1 = nc.dram_tensor("adr1", [128, H * F], bf16, kind="ExternalInput")
    asr2 = nc.dram_tensor("asr2", [128, H * F], bf16, kind="ExternalInput")
    adr2 = nc.dram_tensor("adr2", [128, H * F], bf16, kind="ExternalInput")
    gidx = nc.dram_tensor("gidx", [128, S * 8], i16, kind="ExternalInput")
    identb = nc.dram_tensor("identb", [128, 128], bf16, kind="ExternalInput")
    identf = nc.dram_tensor("identf", [128, 128], f32, kind="ExternalInput")
    sent1 = nc.dram_tensor("sent1", [1, RW], f32, kind="ExternalInput")
    sent2 = nc.dram_tensor("sent2", [1, RW], f32, kind="ExternalInput")
    b1r = nc.dram_tensor("b1r", [128, F], f32, kind="ExternalInput")
    b2r = nc.dram_tensor("b2r", [128, F], f32, kind="ExternalInput")

    T1 = nc.dram_tensor("T1", [TROWS, RW], f32)
    T2 = nc.dram_tensor("T2", [TROWS, RW], f32)
    Ts1 = nc.dram_tensor("Ts1", [PERC, RW], f32)
    Ts2 = nc.dram_tensor("Ts2", [PERC, RW], f32)
    o1T = nc.dram_tensor("o1T", [F, PERC], bf16)
    o1Tg = nc.dram_tensor("o1Tg", [NCORES * F, PERC], bf16, addr_space="Shared")
    out2p = nc.dram_tensor("out2p", [PERC, F], f32, kind="ExternalOutput")

    NC3 = ((MAXC + GRP - 1) // GRP) * GRP  # rhs tile columns (33)

    with TileContext(nc) as tc:
        with (
            tc.tile_pool(name="cons", bufs=1) as cons,
            tc.tile_pool(name="sbA", bufs=3) as sbA,
            tc.tile_pool(name="psA", bufs=2, space="PSUM") as psA,
            tc.tile_pool(name="dp", bufs=2) as dp,
            tc.tile_pool(name="gp", bufs=2) as gp,
            tc.tile_pool(name="tp", bufs=2) as tp,
            tc.tile_pool(name="rp", bufs=2) as rp,
            tc.tile_pool(name="ep", bufs=8) as ep,
            tc.tile_pool(name="psE", bufs=3, space="PSUM") as psE,
            tc.tile_pool(name="psT", bufs=1, space="PSUM") as psT,
        ):
            identb_sb = cons.tile([128, 128], bf16)
            nc.sync.dma_start(out=identb_sb[:], in_=identb[:, :])
            identf_sb = cons.tile([128, 128], f32)
            nc.sync.dma_start(out=identf_sb[:], in_=identf[:, :])
            w1_sb = cons.tile([IN, H * F], bf16)
            nc.sync.dma_start(out=w1_sb[:], in_=w1[:, :])
            w2_sb = cons.tile([F, H * F], bf16)
            nc.sync.dma_start(out=w2_sb[:], in_=w2[:, :])
            asr1_sb = cons.tile([128, H * F], bf16)
            nc.sync.dma_start(out=asr1_sb[:], in_=asr1[:, :])
            adr1_sb = cons.tile([128, H * F], bf16)
            nc.sync.dma_start(out=adr1_sb[:], in_=adr1[:, :])
            asr2_sb = cons.tile([128, H * F], bf16)
            nc.sync.dma_start(out=asr2_sb[:], in_=asr2[:, :])
            adr2_sb = cons.tile([128, H * F], bf16)
            nc.sync.dma_start(out=adr2_sb[:], in_=adr2[:, :])
            b1r_sb = cons.tile([128, F], f32)
            nc.sync.dma_start(out=b1r_sb[:], in_=b1r[:, :])
            b2r_sb = cons.tile([128, F], f32)
            nc.sync.dma_start(out=b2r_sb[:], in_=b2r[:, :])
            sent1_sb = cons.tile([1, RW], f32)
            nc.sync.dma_start(out=sent1_sb[:], in_=sent1[:, :])
            sent2_sb = cons.tile([1, RW], f32)
            nc.sync.dma_start(out=sent2_sb[:], in_=sent2[:, :])
            gidx_sb = cons.tile([128, S * 8], i16)
            nc.sync.dma_start(out=gidx_sb[:], in_=gidx[:, :])

            def edge_layer(tbl, tself, asr_sb, adr_sb, bias_sb, is_layer1):
                tbl_lo = tbl[0:LO_END, :]
                tbl_hi = tbl[HI_START:TROWS, :]
                for b in range(NBLK):
                    w_b = min(128, PERC - b * 128)
                    # ---- self/dst slab read + dot products ----
                    dt = dp.tile([128, RW], f32, tag="dt")
                    if w_b < 128:
                        nc.vector.memset(dt[:], 0.0)
                    nc.sync.dma_start(out=dt[0:w_b, :],
                                      in_=tself[b * 128:b * 128 + w_b, :])
                    dhb = dt[:].bitcast(bf16)          # [128, 128] h-major
                    tmpd = tp.tile([128, 128], bf16, tag="tmpd")
                    nc.vector.tensor_tensor(out=tmpd[:], in0=dhb, in1=asr_sb[:],
                                            op=mybir.AluOpType.mult)
                    asl = ep.tile([128, H], f32, tag="asl")
                    nc.vector.tensor_reduce(
                        out=asl[:], in_=tmpd[:].rearrange("p (h f) -> p h f", f=F),
                        axis=mybir.AxisListType.X, op=mybir.AluOpType.add)
                    nc.vector.tensor_tensor(out=tmpd[:], in0=dhb, in1=adr_sb[:],
                                            op=mybir.AluOpType.mult)
                    adl = ep.tile([128, H], f32, tag="adl")
                    nc.vector.tensor_reduce(
                        out=adl[:], in_=tmpd[:].rearrange("p (h f) -> p h f", f=F),
                        axis=mybir.AxisListType.X, op=mybir.AluOpType.add)
                    # self edge: e = LRelu(asl + adl); w = exp(e)
                    es = ep.tile([128, H], f32, tag="es")
                    nc.vector.tensor_tensor(out=es[:], in0=asl[:], in1=adl[:],
                                            op=mybir.AluOpType.add)
                    es2 = ep.tile([128, H], f32, tag="es2")
                    nc.vector.tensor_scalar(out=es2[:], in0=es[:], scalar1=NEG,
                                            scalar2=None, op0=mybir.AluOpType.mult)
                    nc.vector.tensor_tensor(out=es2[:], in0=es2[:], in1=es[:],
                                            op=mybir.AluOpType.max)
                    wsb = ep.tile([128, H], bf16, tag="wsb")
                    nc.scalar.activation(out=wsb[:], in_=es2[:],
                                         func=mybir.ActivationFunctionType.Exp)
                    rhs_s = ep.tile([128, 4 + H * F], f32, tag="rhs_s")
                    nc.vector.tensor_copy(out=rhs_s[:, 0:4], in_=wsb[:])
                    nc.vector.tensor_tensor(
                        out=rhs_s[:, 4:132].rearrange("p (h f) -> p h f", f=F),
                        in0=dhb.rearrange("p (h f) -> p h f", f=F),
                        in1=wsb[:].unsqueeze(2).to_broadcast([128, H, F]),
                        op=mybir.AluOpType.mult)

                    # ---- gathered slots ----
                    psum = psE.tile([128, GRP * 132], f32, tag="acc")
                    n_tri = sum((cc + GRP - 1) // GRP
                                for nn in (int(nl[b]), int(nh[b]))
                                for cc in _chunks(nn))
                    tri = 0
                    for half in range(2):
                        ncols_all = int(nl[b]) if half == 0 else int(nh[b])
                        col0 = int(meta["col_lo"][b]) if half == 0 \
                            else int(meta["col_hi"][b])
                        view = tbl_lo if half == 0 else tbl_hi
                        for s0 in range(0, ncols_all, MAXC):
                            ncc = min(MAXC, ncols_all - s0)
                            nc3 = ((ncc + GRP - 1) // GRP) * GRP
                            g = gp.tile([128, MAXC, RW], f32, tag="g")
                            nc.gpsimd.dma_gather(
                                g[:, 0:ncc, :], view,
                                gidx_sb[:, (col0 + s0) * 8:(col0 + s0 + ncc) * 8],
                                ncc * 128, ncc * 128, RW,
                                single_packet=(ncc * 128 <= 1008))
                            gb = g[:].bitcast(bf16)    # [128, MAXC, 128]
                            tmp = tp.tile([128, MAXC, 128], bf16, tag="tmp")
                            nc.vector.tensor_tensor(
                                out=tmp[:, 0:ncc, :], in0=gb[:, 0:ncc, :],
                                in1=asr_sb[:].unsqueeze(1).to_broadcast(
                                    [128, ncc, H * F]),
                                op=mybir.AluOpType.mult)
                            al = ep.tile([128, MAXC, H], f32, tag="al")
                            nc.vector.tensor_reduce(
                                out=al[:, 0:ncc, :],
                                in_=tmp[:, 0:ncc, :].rearrange(
                                    "p n (h f) -> p n h f", f=F),
                                axis=mybir.AxisListType.X,
                                op=mybir.AluOpType.add)
                            nc.vector.tensor_tensor(
                                out=al[:, 0:ncc, :], in0=al[:, 0:ncc, :],
                                in1=adl[:].unsqueeze(1).to_broadcast(
                                    [128, ncc, H]),
                                op=mybir.AluOpType.add)
                            e2 = ep.tile([128, MAXC, H], f32, tag="e2")
                            nc.vector.tensor_scalar(
                                out=e2[:, 0:ncc, :], in0=al[:, 0:ncc, :],
                                scalar1=NEG, scalar2=None,
                                op0=mybir.AluOpType.mult)
                            nc.vector.tensor_tensor(
                                out=e2[:, 0:ncc, :], in0=e2[:, 0:ncc, :],
                                in1=al[:, 0:ncc, :], op=mybir.AluOpType.max)
                            rhs = rp.tile([128, NC3, 132], bf16, tag="rhs")
                            if nc3 > ncc:
                                nc.vector.memset(rhs[:, ncc:nc3, :], 0.0)
                            nc.scalar.activation(
                                out=rhs[:, 0:ncc, 0:4], in_=e2[:, 0:ncc, :],
                                func=mybir.ActivationFunctionType.Exp)
                            nc.vector.tensor_tensor(
                                out=rhs[:, 0:ncc, 4:132].rearrange(
                                    "p n (h f) -> p n h f", f=F),
                                in0=gb[:, 0:ncc, :].rearrange(
                                    "p n (h f) -> p n h f", f=F),
                                in1=rhs[:, 0:ncc, 0:4].unsqueeze(3).to_broadcast(
                                    [128, ncc, H, F]),
                                op=mybir.AluOpType.mult)
                            for t in range(nc3 // GRP):
                                nc.tensor.matmul(
                                    out=psum[:],
                                    lhsT=identb_sb[:],
                                    rhs=rhs[:, t * GRP:(t + 1) * GRP, :].rearrange(
                                        "p a b -> p (a b)"),
                                    start=(tri == 0), stop=(tri == n_tri - 1))
                                tri += 1
                    assert tri == n_tri

                    # ---- epilogue ----
                    U = ep.tile([128, 132], f32, tag="U")
                    nc.vector.tensor_tensor(out=U[:], in0=rhs_s[:],
                                            in1=psum[:, 0:132],
                                            op=mybir.AluOpType.add)
                    nc.vector.tensor_tensor(out=U[:], in0=U[:],
                                            in1=psum[:, 132:264],
                                            op=mybir.AluOpType.add)
                    nc.vector.tensor_tensor(out=U[:], in0=U[:],
                                            in1=psum[:, 264:396],
                                            op=mybir.AluOpType.add)
                    sden = ep.tile([128, H], f32, tag="sden")
                    nc.vector.tensor_scalar(out=sden[:], in0=U[:, 0:4],
                                            scalar1=1e-16, scalar2=None,
                                            op0=mybir.AluOpType.add)
                    rv = ep.tile([128, H], f32, tag="rv")
                    nc.vector.reciprocal(out=rv[:], in_=sden[:])
                    nc.vector.tensor_scalar(out=rv[:], in0=rv[:], scalar1=1.0 / H,
                                            scalar2=None,
                                            op0=mybir.AluOpType.mult)
                    m = ep.tile([128, H * F], f32, tag="m")
                    nc.vector.tensor_tensor(
                        out=m[:].rearrange("p (h f) -> p h f", f=F),
                        in0=U[:, 4:132].rearrange("p (h f) -> p h f", f=F),
                        in1=rv[:].unsqueeze(2).to_broadcast([128, H, F]),
                        op=mybir.AluOpType.mult)
                    o = ep.tile([128, F], f32, tag="o")
                    nc.vector.tensor_tensor(out=o[:], in0=m[:, 0:F],
                                            in1=m[:, F:2 * F],
                                            op=mybir.AluOpType.add)
                    o2 = ep.tile([128, F], f32, tag="o2t")
                    nc.vector.tensor_tensor(out=o2[:], in0=m[:, 2 * F:3 * F],
                                            in1=m[:, 3 * F:4 * F],
                                            op=mybir.AluOpType.add)
                    nc.vector.tensor_tensor(out=o[:], in0=o[:], in1=o2[:],
                                            op=mybir.AluOpType.add)
                    nc.vector.tensor_tensor(out=o[:], in0=o[:], in1=bias_sb[:],
                                            op=mybir.AluOpType.add)
                    if is_layer1:
                        # ELU
                        m0 = ep.tile([128, F], f32, tag="m0")
                        nc.vector.tensor_scalar(out=m0[:], in0=o[:], scalar1=0.0,
                                                scalar2=None,
                                                op0=mybir.AluOpType.min)
                        em = ep.tile([128, F], f32, tag="em")
                        nc.scalar.activation(out=em[:], in_=m0[:],
                                             func=mybir.ActivationFunctionType.Exp)
                        nc.vector.tensor_scalar(out=em[:], in0=em[:], scalar1=-1.0,
                                                scalar2=None,
                                                op0=mybir.AluOpType.add)
                        nc.vector.tensor_tensor(out=o[:], in0=o[:], in1=em[:],
                                                op=mybir.AluOpType.max)
                        # transpose -> o1T (bf16) + h2 = o1 @ W2 -> Ts2
                        pT = psT.tile([F, 128], f32, tag="pT")
                        nc.tensor.transpose(out=pT[:], in_=o[:],
                                            identity=identf_sb[:])
                        oT = ep.tile([F, 128], bf16, tag="oT")
                        nc.vector.tensor_copy(out=oT[:], in_=pT[:])
                        nc.sync.dma_start(out=o1T[:, b * 128:b * 128 + w_b],
                                          in_=oT[:, 0:w_b])
                        ps2 = psT.tile([128, H * F], f32, tag="ps2")
                        nc.tensor.matmul(out=ps2[:], lhsT=oT[:], rhs=w2_sb[:],
                                         start=True, stop=True)
                        st2 = ep.tile([128, RW], f32, tag="st2")
                        nc.vector.tensor_copy(out=st2[:].bitcast(bf16),
                                              in_=ps2[:])
                        nc.sync.dma_start(out=Ts2[b * 128:b * 128 + w_b, :],
                                          in_=st2[0:w_b, :])
                    else:
                        nc.sync.dma_start(out=out2p[b * 128:b * 128 + w_b, :],
                                          in_=o[0:w_b, :])

            # ---- stage A, layer 1 (full table, replicated) + self slab ----
            SL = 16 * 128
            views1 = [(xTg[:, s0:min(s0 + SL, TROWS)], s0)
                      for s0 in range(0, TROWS, SL)]
            _stage_a(nc, (sbA, psA), views1, w1_sb, T1, IN)
            nc.sync.dma_start(out=T1[SENT_ROW:SENT_ROW + 1, :], in_=sent1_sb[:])
            viewsS = [(xTs[:, s0:min(s0 + SL, PERC)], s0)
                      for s0 in range(0, PERC, SL)]
            _stage_a(nc, (sbA, psA), viewsS, w1_sb, Ts1, IN)

            # ---- layer 1 edges ----
            edge_layer(T1, Ts1, asr1_sb, adr1_sb, b1r_sb, True)

            # ---- allgather o1T ----
            nc.gpsimd.collective_compute(
                "AllGather", mybir.AluOpType.bypass,
                replica_groups=[list(range(NCORES))],
                ins=[o1T[:].opt()], outs=[o1Tg[:].opt()])

            # ---- stage A, layer 2 (replicated from o1Tg) ----
            views2 = []
            for r in range(NCORES):
                for p0 in range(0, PERC, SL):
                    views2.append((o1Tg[r * F:(r + 1) * F, p0:min(p0 + SL, PERC)],
                                   r * SLAB + p0))
            _stage_a(nc, (sbA, psA), views2, w2_sb, T2, F)
            nc.sync.dma_start(out=T2[SENT_ROW:SENT_ROW + 1, :], in_=sent2_sb[:])

            # ---- layer 2 edges ----
            edge_layer(T2, Ts2, asr2_sb, adr2_sb, b2r_sb, False)

    nc.compile()
    return nc


_CACHE = {}


def _prepare(x, edge_index, W1, att_src1, att_dst1, b1, W2, att_src2,
             att_dst2, b2):
    x = np.asarray(x, np.float32)
    edge_index = np.asarray(edge_index, np.int64)
    key = hash(edge_index.tobytes())
    if key in _CACHE:
        meta, nc = _CACHE[key]
    else:
        meta = _preprocess(edge_index)
        nc = _build_program(meta)
        _CACHE[key] = (meta, nc)

    bf = ml_dtypes.bfloat16
    W1b = np.asarray(W1, np.float32).astype(bf)
    W2b = np.asarray(W2, np.float32).astype(bf)

    def att_rep(a):
        return np.broadcast_to(
            np.asarray(a, np.float32).reshape(H * F).astype(bf), (128, H * F)
        ).copy()

    def sent_row(a_src):
        Afull = np.zeros((H, H * F))
        a = np.asarray(a_src, np.float64)
        for h in range(H):
            Afull[h, h * F:(h + 1) * F] = a[h]
        v, *_ = np.linalg.lstsq(Afull, -300.0 * np.ones(H), rcond=None)
        return np.ascontiguousarray(v.astype(bf)).view(np.float32).reshape(1, RW)

    # x columns in g-order (junk cols zero)
    xb = x.astype(bf)
    arr = np.zeros((TROWS, IN), bf)
    arr[meta["row_of"]] = xb
    xTg = np.ascontiguousarray(arr.T)

    common = dict(
        xTg=xTg, w1=W1b, w2=W2b,
        asr1=att_rep(att_src1), adr1=att_rep(att_dst1),
        asr2=att_rep(att_src2), adr2=att_rep(att_dst2),
        identb=np.eye(128, dtype=bf), identf=np.eye(128, dtype=np.float32),
        sent1=sent_row(att_src1), sent2=sent_row(att_src2),
        b1r=np.broadcast_to(np.asarray(b1, np.float32), (128, F)).copy(),
        b2r=np.broadcast_to(np.asarray(b2, np.float32), (128, F)).copy(),
    )
    in_maps = []
    for c in range(NCORES):
        xTs = np.ascontiguousarray(xb[meta["perms"][c]].T)
        in_maps.append(dict(common, gidx=meta["gidx"][c], xTs=xTs))
    return nc, in_maps, meta


def _assemble(meta, results):
    out = np.empty((N, F), np.float32)
    for c in range(NCORES):
        out[meta["perms"][c]] = results[c]["out2p"]
    return out


def kernel(**inputs):
    nc, in_maps, meta = _prepare(**inputs)
    res = run_bass_kernel_spmd(nc, in_maps, core_ids=list(range(NCORES)))
    return _assemble(meta, res.results)


def run_traced(**inputs):
    """Profiled run; returns BassKernelResults (exec_time_ns etc.)."""
    nc, in_maps, meta = _prepare(**inputs)
    res = run_bass_kernel_spmd(nc, in_maps, core_ids=list(range(NCORES)),
                               trace=True)
    res.gat_output = _assemble(meta, res.results)
    return res


# revision 16
# speedup vs baseline: 1.4427x; 1.4427x over previous
"""2-layer GAT (PyG GATConv, concat=False, self-loops) on 8 Trainium2 cores.

Design (v2): nodes/edges partitioned by destination across 8 cores; each
core owns a 6250-dst range, dsts degree-sorted into 49 blocks of 128 PSUM
lanes.  Per-edge source features are fetched with dma_gather from a
256B-per-node table in HBM (pure bf16 h, h-major); per-edge attention
logits are recomputed on-chip with a DVE dot against a replicated a_src
row, so rows stay at the 256B dma_gather minimum.  The node table lives in
g-space (8 slabs of 6251 rows, one junk row per slab); one int16 index
stream serves both layers.  int16 gather indices cover 32767 rows, so
edges are split between two overlapping views (lo=[0,32767),
hi=[17241,50008)) with per-lane balancing of the 31% of sources that both
views can address.  Self-loop edges are excluded from the gather: each
core streams its own dst rows sequentially (also providing a_dst) and adds
the self contribution in the epilogue.  Padding slots point at a sentinel
row v with a_src@v = -300 so exp(LRelu(...)) ~ 0.  Scatter is free: slots
accumulate into PSUM via identity-weight matmuls (3 columns per matmul).
Layer boundary: AllGather of the bf16-transposed layer-1 output, then a
replicated stage-A rebuilds the layer-2 table on every core.
"""
import sys
sys.path.insert(0, "/opt/trn_rl_repo")

import numpy as np
import ml_dtypes

import concourse.bass as bass
import concourse.bacc as bacc
import concourse.mybir as mybir
from concourse.bass_utils import run_bass_kernel_spmd
from concourse.tile import TileContext

N = 50000
E = 1600000
IN = 128
H = 4
F = 32
NEG = 0.2
NCORES = 8
PERC = N // NCORES           # 6250
SLAB = PERC + 1              # 6251 (junk row at pos 6250)
TROWS = NCORES * SLAB        # 50008
NBLK = (PERC + 127) // 128   # 49
LO_END = 32767               # lo view = rows [0, 32767)
HI_START = TROWS - 32767     # 17241; hi view = rows [17241, 50008)
SENT_ROW = 3 * SLAB + PERC   # 25003 (core 3's junk row; inside the overlap)
RW = 64                      # table row: 64 f32 slots = 256B (128 bf16 h)
MAXC = int(__import__('os').environ.get('GAT_MAXC', '15'))  # max gather columns per dma_gather call
GRP = 3                      # aggregation matmul: 3 slot-columns per matmul


def _chunks(n):
    return [min(MAXC, n - s) for s in range(0, n, MAXC)]


def _pack_idx(idx_flat):
    """[n] -> [128, n/16] int16; idx i -> (partition i%16, col i//16), x8."""
    n = idx_flat.shape[0]
    assert n % 16 == 0
    a = idx_flat.reshape(n // 16, 16).T.astype(np.int16)
    return np.ascontiguousarray(np.tile(a, (8, 1)))


def _preprocess(edge_index):
    """Index preprocessing. Returns per-core gidx + shared structure."""
    src0 = edge_index[0].astype(np.int64)
    dst0 = edge_index[1].astype(np.int64)
    deg = np.bincount(dst0, minlength=N)          # gather-degree (no self)

    def build_perms(lo_cnt):
        perms = []
        pos_of = np.empty(N, np.int64)
        for c in range(NCORES):
            ids = np.arange(c * PERC, (c + 1) * PERC)
            order = np.lexsort((-lo_cnt[ids], -(deg[ids] // 4)))
            perm = ids[order]
            perms.append(perm)
            pos_of[perm] = np.arange(PERC)
        return perms, pos_of

    # pass 1: degree only -> provisional rows -> per-dst must-lo counts
    perms, pos_of = build_perms(np.zeros(N, np.int64))
    for _ in range(2):
        row_of = (np.arange(N) // PERC) * SLAB + pos_of
        must_lo = row_of[src0] < HI_START
        lo_cnt = np.bincount(dst0[must_lo], minlength=N)
        perms, pos_of = build_perms(lo_cnt)

    row_of = (np.arange(N) // PERC) * SLAB + pos_of
    r_src = row_of[src0]
    kind = np.where(r_src < HI_START, 0, np.where(r_src >= LO_END, 1, 2))

    lane_all = pos_of[dst0]
    core_all = dst0 // PERC
    blk_all = lane_all // 128

    # per-block slot counts nl/nh (cross-core maxima)
    A = np.zeros(NBLK, np.int64)
    B = np.zeros(NBLK, np.int64)
    C = np.zeros(NBLK, np.int64)
    cnt_l = np.zeros((NCORES, PERC), np.int64)
    cnt_h = np.zeros((NCORES, PERC), np.int64)
    cnt_f = np.zeros((NCORES, PERC), np.int64)
    for c in range(NCORES):
        m = core_all == c
        lane = lane_all[m]
        k = kind[m]
        cnt_l[c] = np.bincount(lane[k == 0], minlength=PERC)
        cnt_h[c] = np.bincount(lane[k == 1], minlength=PERC)
        cnt_f[c] = np.bincount(lane[k == 2], minlength=PERC)
    for b in range(NBLK):
        sl = slice(b * 128, min((b + 1) * 128, PERC))
        A[b] = cnt_l[:, sl].max()
        B[b] = cnt_h[:, sl].max()
        C[b] = (cnt_l[:, sl] + cnt_h[:, sl] + cnt_f[:, sl]).max()
    nl = A.copy()
    nh = np.maximum(B, C - A)
    S = int((nl + nh).sum())
    col_lo = np.zeros(NBLK, np.int64)   # column start of block's lo run
    col_hi = np.zeros(NBLK, np.int64)
    col = 0
    for b in range(NBLK):
        col_lo[b] = col
        col += nl[b]
        col_hi[b] = col
        col += nh[b]
    assert col == S

    # per-core column fill
    gidx = []
    sent_lo, sent_hi = SENT_ROW, SENT_ROW - HI_START
    for c in range(NCORES):
        m = core_all == c
        lane = lane_all[m]
        k = kind[m].copy()
        rows = r_src[m]
        # flex -> lo for the first a_i of each lane, rest -> hi
        nh_of_lane = nh[np.arange(PERC) // 128]
        a_i = np.maximum(0, cnt_h[c] + cnt_f[c] - nh_of_lane)
        # order edges by (lane, kind-rank, row); flex edges get rank by
        # position so the first a_i go to lo
        order = np.lexsort((rows, k, lane))
        lane_s, k_s, rows_s = lane[order], k[order], rows[order]
        # cumcount within (lane, kind)
        start = np.r_[True, (lane_s[1:] != lane_s[:-1]) | (k_s[1:] != k_s[:-1])]
        grp_id = np.cumsum(start) - 1
        first_pos = np.full(grp_id[-1] + 1, 1 << 62, np.int64)
        np.minimum.at(first_pos, grp_id, np.arange(len(grp_id)))
        cum = np.arange(len(grp_id)) - first_pos[grp_id]
        # stream + slot per edge
        is_lo = (k_s == 0) | ((k_s == 2) & (cum < a_i[lane_s]))
        slot = np.where(
            k_s == 0, cum,                                   # must-lo
            np.where(k_s == 1, cum,                          # must-hi
                     np.where(is_lo, cnt_l[c][lane_s] + cum,  # flex->lo
                              cnt_h[c][lane_s] + cum - a_i[lane_s])))  # flex->hi
        b_s = lane_s // 128
        colidx = np.where(is_lo, col_lo[b_s] + slot, col_hi[b_s] + slot)
        grid = np.full((S, 128), -1, np.int64)
        grid[colidx, lane_s % 128] = rows_s
        # sentinels + view-relative index
        lo_cols = np.zeros(S, bool)
        for b in range(NBLK):
            lo_cols[col_lo[b]:col_lo[b] + nl[b]] = True
        grid[lo_cols] = np.where(grid[lo_cols] < 0, sent_lo, grid[lo_cols])
        grid[~lo_cols] = np.where(grid[~lo_cols] < 0, sent_hi + HI_START,
                                  grid[~lo_cols]) - HI_START
        assert grid.min() >= 0 and grid.max() < LO_END
        gidx.append(_pack_idx(grid.reshape(-1)))

    return dict(gidx=gidx, nl=nl, nh=nh, S=S, col_lo=col_lo, col_hi=col_hi,
                perms=perms, pos_of=pos_of, row_of=row_of)


def _stage_a(nc, pools, views, w_sb, tbl, kdim):
    """h = lhsT.T @ W -> bf16 rows packed into tbl.

    views: list of (lhsT_view [kdim, W], row0); W <= 16*128. Rows go out as
    slab DMAs to tbl[row0:row0+W] (f32 [*, 64] holding 128 bf16).
    """
    sb, ps = pools
    f32, bf16 = mybir.dt.float32, mybir.dt.bfloat16
    for (view, row0) in views:
        Wt = view.shape[1]
        nt = (Wt + 127) // 128
        xsb = sb.tile([kdim, 16 * 128], bf16, tag="xa")
        nc.sync.dma_start(out=xsb[:, 0:Wt], in_=view)
        stg = sb.tile([128, 16, RW], f32, tag="sa")
        for t in range(nt):
            w = min(128, Wt - t * 128)
            psum = ps.tile([128, 128], f32, tag="pa")
            nc.tensor.matmul(out=psum[0:w, :], lhsT=xsb[:, t * 128:t * 128 + w],
                             rhs=w_sb[:], start=True, stop=True)
            nc.vector.tensor_copy(
                out=stg[0:w, t, :].bitcast(bf16),
                in_=psum[0:w, :])
        nfull = Wt // 128
        if nfull:
            nc.sync.dma_start(
                out=tbl[row0:row0 + nfull * 128, :].rearrange(
                    "(t p) c -> p t c", p=128),
                in_=stg[:, 0:nfull, :])
        if Wt % 128:
            nc.sync.dma_start(out=tbl[row0 + nfull * 128:row0 + Wt, :],
                              in_=stg[0:Wt % 128, nfull, :])


def _build_program(meta):
    nl, nh, S = meta["nl"], meta["nh"], meta["S"]
    nc = bacc.Bacc("TRN2", target_bir_lowering=False, debug=False,
                   num_devices=NCORES, dynamic_dma_scratch_size=int(__import__('os').environ.get('GAT_SCRATCH', '65536')))

    f32, bf16, i16 = mybir.dt.float32, mybir.dt.bfloat16, mybir.dt.int16
    xTg = nc.dram_tensor("xTg", [IN, TROWS], bf16, kind="ExternalInput")
    xTs = nc.dram_tensor("xTs", [IN, PERC], bf16, kind="ExternalInput")
    w1 = nc.dram_tensor("w1", [IN, H * F], bf16, kind="ExternalInput")
    w2 = nc.dram_tensor("w2", [F, H * F], bf16, kind="ExternalInput")
    asr1 = nc.dram_tensor("asr1", [128, H * F], bf16, kind="ExternalInput")
    adr1 = nc.dram_tensor("adr1", [128, H * F], bf16, kind="ExternalInput")
    asr2 = nc.dram_tensor("asr2", [128, H * F], bf16, kind="ExternalInput")
    adr2 = nc.dram_tensor("adr2", [128, H * F], bf16, kind="ExternalInput")
    gidx = nc.dram_tensor("gidx", [128, S * 8], i16, kind="ExternalInput")
    identb = nc.dram_tensor("identb", [128, 128], bf16, kind="ExternalInput")
    identf = nc.dram_tensor("identf", [128, 128], f32, kind="ExternalInput")
    sent1 = nc.dram_tensor("sent1", [1, RW], f32, kind="ExternalInput")
    sent2 = nc.dram_tensor("sent2", [1, RW], f32, kind="ExternalInput")
    b1r = nc.dram_tensor("b1r", [128, F], f32, kind="ExternalInput")
    b2r = nc.dram_tensor("b2r", [128, F], f32, kind="ExternalInput")

    T1 = nc.dram_tensor("T1", [TROWS, RW], f32)
    T2 = nc.dram_tensor("T2", [TROWS, RW], f32)
    Ts1 = nc.dram_tensor("Ts1", [PERC, RW], f32)
    Ts2 = nc.dram_tensor("Ts2", [PERC, RW], f32)
    o1T = nc.dram_tensor("o1T", [F, PERC], bf16)
    o1Tg = nc.dram_tensor("o1Tg", [NCORES * F, PERC], bf16, addr_space="Shared")
    out2p = nc.dram_tensor("out2p", [PERC, F], f32, kind="ExternalOutput")

    NC3 = ((MAXC + GRP - 1) // GRP) * GRP  # rhs tile columns (33)

    with TileContext(nc) as tc:
        with (
            tc.tile_pool(name="cons", bufs=1) as cons,
            tc.tile_pool(name="sbA", bufs=3) as sbA,
            tc.tile_pool(name="psA", bufs=2, space="PSUM") as psA,
            tc.tile_pool(name="dp", bufs=2) as dp,
            tc.tile_pool(name="gp", bufs=4) as gp,
            tc.tile_pool(name="tp", bufs=3) as tp,
            tc.tile_pool(name="rp", bufs=3) as rp,
            tc.tile_pool(name="ep", bufs=8) as ep,
            tc.tile_pool(name="psE", bufs=3, space="PSUM") as psE,
            tc.tile_pool(name="psT", bufs=1, space="PSUM") as psT,
        ):
            identb_sb = cons.tile([128, 128], bf16)
            nc.sync.dma_start(out=identb_sb[:], in_=identb[:, :])
            identf_sb = cons.tile([128, 128], f32)
            nc.sync.dma_start(out=identf_sb[:], in_=identf[:, :])
            w1_sb = cons.tile([IN, H * F], bf16)
            nc.sync.dma_start(out=w1_sb[:], in_=w1[:, :])
            w2_sb = cons.tile([F, H * F], bf16)
            nc.sync.dma_start(out=w2_sb[:], in_=w2[:, :])
            asr1_sb = cons.tile([128, H * F], bf16)
            nc.sync.dma_start(out=asr1_sb[:], in_=asr1[:, :])
            adr1_sb = cons.tile([128, H * F], bf16)
            nc.sync.dma_start(out=adr1_sb[:], in_=adr1[:, :])
            asr2_sb = cons.tile([128, H * F], bf16)
            nc.sync.dma_start(out=asr2_sb[:], in_=asr2[:, :])
            adr2_sb = cons.tile([128, H * F], bf16)
            nc.sync.dma_start(out=adr2_sb[:], in_=adr2[:, :])
            b1r_sb = cons.tile([128, F], f32)
            nc.sync.dma_start(out=b1r_sb[:], in_=b1r[:, :])
            b2r_sb = cons.tile([128, F], f32)
            nc.sync.dma_start(out=b2r_sb[:], in_=b2r[:, :])
            sent1_sb = cons.tile([1, RW], f32)
            nc.sync.dma_start(out=sent1_sb[:], in_=sent1[:, :])
            sent2_sb = cons.tile([1, RW], f32)
            nc.sync.dma_start(out=sent2_sb[:], in_=sent2[:, :])
            gidx_sb = cons.tile([128, S * 8], i16)
            nc.sync.dma_start(out=gidx_sb[:], in_=gidx[:, :])

            def edge_layer(tbl, tself, asr_sb, adr_sb, bias_sb, is_layer1):
                tbl_lo = tbl[0:LO_END, :]
                tbl_hi = tbl[HI_START:TROWS, :]
                for b in range(NBLK):
                    w_b = min(128, PERC - b * 128)
                    # ---- self/dst slab read + dot products ----
                    dt = dp.tile([128, RW], f32, tag="dt")
                    if w_b < 128:
                        nc.vector.memset(dt[:], 0.0)
                    nc.sync.dma_start(out=dt[0:w_b, :],
                                      in_=tself[b * 128:b * 128 + w_b, :])
                    dhb = dt[:].bitcast(bf16)          # [128, 128] h-major
                    tmpd = tp.tile([128, 128], bf16, tag="tmpd")
                    nc.vector.tensor_tensor(out=tmpd[:], in0=dhb, in1=asr_sb[:],
                                            op=mybir.AluOpType.mult)
                    asl = ep.tile([128, H], f32, tag="asl")
                    nc.vector.tensor_reduce(
                        out=asl[:], in_=tmpd[:].rearrange("p (h f) -> p h f", f=F),
                        axis=mybir.AxisListType.X, op=mybir.AluOpType.add)
                    nc.vector.tensor_tensor(out=tmpd[:], in0=dhb, in1=adr_sb[:],
                                            op=mybir.AluOpType.mult)
                    adl = ep.tile([128, H], f32, tag="adl")
                    nc.vector.tensor_reduce(
                        out=adl[:], in_=tmpd[:].rearrange("p (h f) -> p h f", f=F),
                        axis=mybir.AxisListType.X, op=mybir.AluOpType.add)
                    # self edge: e = LRelu(asl + adl); w = exp(e)
                    es = ep.tile([128, H], f32, tag="es")
                    nc.vector.tensor_tensor(out=es[:], in0=asl[:], in1=adl[:],
                                            op=mybir.AluOpType.add)
                    es2 = ep.tile([128, H], f32, tag="es2")
                    nc.vector.tensor_scalar(out=es2[:], in0=es[:], scalar1=NEG,
                                            scalar2=None, op0=mybir.AluOpType.mult)
                    nc.vector.tensor_tensor(out=es2[:], in0=es2[:], in1=es[:],
                                            op=mybir.AluOpType.max)
                    wsb = ep.tile([128, H], bf16, tag="wsb")
                    nc.scalar.activation(out=wsb[:], in_=es2[:],
                                         func=mybir.ActivationFunctionType.Exp)
                    rhs_s = ep.tile([128, 4 + H * F], f32, tag="rhs_s")
                    nc.vector.tensor_copy(out=rhs_s[:, 0:4], in_=wsb[:])
                    nc.vector.tensor_tensor(
                        out=rhs_s[:, 4:132].rearrange("p (h f) -> p h f", f=F),
                        in0=dhb.rearrange("p (h f) -> p h f", f=F),
                        in1=wsb[:].unsqueeze(2).to_broadcast([128, H, F]),
                        op=mybir.AluOpType.mult)

                    # ---- gathered slots ----
                    psum = psE.tile([128, GRP * 132], f32, tag="acc")
                    n_tri = sum((cc + GRP - 1) // GRP
                                for nn in (int(nl[b]), int(nh[b]))
                                for cc in _chunks(nn))
                    tri = 0
                    for half in range(2):
                        ncols_all = int(nl[b]) if half == 0 else int(nh[b])
                        col0 = int(meta["col_lo"][b]) if half == 0 \
                            else int(meta["col_hi"][b])
                        view = tbl_lo if half == 0 else tbl_hi
                        for s0 in range(0, ncols_all, MAXC):
                            ncc = min(MAXC, ncols_all - s0)
                            nc3 = ((ncc + GRP - 1) // GRP) * GRP
                            g = gp.tile([128, MAXC, RW], f32, tag="g")
                            nc.gpsimd.dma_gather(
                                g[:, 0:ncc, :], view,
                                gidx_sb[:, (col0 + s0) * 8:(col0 + s0 + ncc) * 8],
                                ncc * 128, ncc * 128, RW,
                                single_packet=(ncc * 128 <= 1008))
                            gb = g[:].bitcast(bf16)    # [128, MAXC, 128]
                            tmp = tp.tile([128, MAXC, 128], bf16, tag="tmp")
                            nc.vector.tensor_tensor(
                                out=tmp[:, 0:ncc, :], in0=gb[:, 0:ncc, :],
                                in1=asr_sb[:].unsqueeze(1).to_broadcast(
                                    [128, ncc, H * F]),
                                op=mybir.AluOpType.mult)
                            al = ep.tile([128, MAXC, H], f32, tag="al")
                            nc.vector.tensor_reduce(
                                out=al[:, 0:ncc, :],
                                in_=tmp[:, 0:ncc, :].rearrange(
                                    "p n (h f) -> p n h f", f=F),
                                axis=mybir.AxisListType.X,
                                op=mybir.AluOpType.add)
                            nc.vector.tensor_tensor(
                                out=al[:, 0:ncc, :], in0=al[:, 0:ncc, :],
                                in1=adl[:].unsqueeze(1).to_broadcast(
                                    [128, ncc, H]),
                                op=mybir.AluOpType.add)
                            alf = al[:, 0:ncc, :].rearrange("p n h -> p (n h)")
                            e2 = ep.tile([128, MAXC, H], f32, tag="e2")
                            e2f = e2[:, 0:ncc, :].rearrange("p n h -> p (n h)")
                            nc.vector.tensor_scalar(
                                out=e2f, in0=alf, scalar1=NEG, scalar2=None,
                                op0=mybir.AluOpType.mult)
                            nc.vector.tensor_tensor(
                                out=e2f, in0=e2f, in1=alf,
                                op=mybir.AluOpType.max)
                            rhs = rp.tile([128, NC3, 132], bf16, tag="rhs")
                            if nc3 > ncc:
                                nc.vector.memset(rhs[:, ncc:nc3, :], 0.0)
                            nc.scalar.activation(
                                out=rhs[:, 0:ncc, 0:4], in_=e2[:, 0:ncc, :],
                                func=mybir.ActivationFunctionType.Exp)
                            nc.vector.tensor_tensor(
                                out=rhs[:, 0:ncc, 4:132].rearrange(
                                    "p n (h f) -> p n h f", f=F),
                                in0=gb[:, 0:ncc, :].rearrange(
                                    "p n (h f) -> p n h f", f=F),
                                in1=rhs[:, 0:ncc, 0:4].unsqueeze(3).to_broadcast(
                                    [128, ncc, H, F]),
                                op=mybir.AluOpType.mult)
                            for t in range(nc3 // GRP):
                                nc.tensor.matmul(
                                    out=psum[:],
                                    lhsT=identb_sb[:],
                                    rhs=rhs[:, t * GRP:(t + 1) * GRP, :].rearrange(
                                        "p a b -> p (a b)"),
                                    start=(tri == 0), stop=(tri == n_tri - 1))
                                tri += 1
                    assert tri == n_tri

                    # ---- epilogue ----
                    U = ep.tile([128, 132], f32, tag="U")
                    nc.vector.tensor_tensor(out=U[:], in0=rhs_s[:],
                                            in1=psum[:, 0:132],
                                            op=mybir.AluOpType.add)
                    nc.vector.tensor_tensor(out=U[:], in0=U[:],
                                            in1=psum[:, 132:264],
                                            op=mybir.AluOpType.add)
                    nc.vector.tensor_tensor(out=U[:], in0=U[:],
                                            in1=psum[:, 264:396],
                                            op=mybir.AluOpType.add)
                    sden = ep.tile([128, H], f32, tag="sden")
                    nc.vector.tensor_scalar(out=sden[:], in0=U[:, 0:4],
                                            scalar1=1e-16, scalar2=None,
                                            op0=mybir.AluOpType.add)
                    rv = ep.tile([128, H], f32, tag="rv")
                    nc.vector.reciprocal(out=rv[:], in_=sden[:])
                    nc.vector.tensor_scalar(out=rv[:], in0=rv[:], scalar1=1.0 / H,
                                            scalar2=None,
                                            op0=mybir.AluOpType.mult)
                    m = ep.tile([128, H * F], f32, tag="m")
                    nc.vector.tensor_tensor(
                        out=m[:].rearrange("p (h f) -> p h f", f=F),
                        in0=U[:, 4:132].rearrange("p (h f) -> p h f", f=F),
                        in1=rv[:].unsqueeze(2).to_broadcast([128, H, F]),
                        op=mybir.AluOpType.mult)
                    o = ep.tile([128, F], f32, tag="o")
                    nc.vector.tensor_tensor(out=o[:], in0=m[:, 0:F],
                                            in1=m[:, F:2 * F],
                                            op=mybir.AluOpType.add)
                    o2 = ep.tile([128, F], f32, tag="o2t")
                    nc.vector.tensor_tensor(out=o2[:], in0=m[:, 2 * F:3 * F],
                                            in1=m[:, 3 * F:4 * F],
                                            op=mybir.AluOpType.add)
                    nc.vector.tensor_tensor(out=o[:], in0=o[:], in1=o2[:],
                                            op=mybir.AluOpType.add)
                    nc.vector.tensor_tensor(out=o[:], in0=o[:], in1=bias_sb[:],
                                            op=mybir.AluOpType.add)
                    if is_layer1:
                        # ELU
                        m0 = ep.tile([128, F], f32, tag="m0")
                        nc.vector.tensor_scalar(out=m0[:], in0=o[:], scalar1=0.0,
                                                scalar2=None,
                                                op0=mybir.AluOpType.min)
                        em = ep.tile([128, F], f32, tag="em")
                        nc.scalar.activation(out=em[:], in_=m0[:],
                                             func=mybir.ActivationFunctionType.Exp)
                        nc.vector.tensor_scalar(out=em[:], in0=em[:], scalar1=-1.0,
                                                scalar2=None,
                                                op0=mybir.AluOpType.add)
                        nc.vector.tensor_tensor(out=o[:], in0=o[:], in1=em[:],
                                                op=mybir.AluOpType.max)
                        # transpose -> o1T (bf16) + h2 = o1 @ W2 -> Ts2
                        pT = psT.tile([F, 128], f32, tag="pT")
                        nc.tensor.transpose(out=pT[:], in_=o[:],
                                            identity=identf_sb[:])
                        oT = ep.tile([F, 128], bf16, tag="oT")
                        nc.vector.tensor_copy(out=oT[:], in_=pT[:])
                        nc.sync.dma_start(out=o1T[:, b * 128:b * 128 + w_b],
                                          in_=oT[:, 0:w_b])
                        ps2 = psT.tile([128, H * F], f32, tag="ps2")
                        nc.tensor.matmul(out=ps2[:], lhsT=oT[:], rhs=w2_sb[:],
                                         start=True, stop=True)
                        st2 = ep.tile([128, RW], f32, tag="st2")
                        nc.vector.tensor_copy(out=st2[:].bitcast(bf16),
                                              in_=ps2[:])
                        nc.sync.dma_start(out=Ts2[b * 128:b * 128 + w_b, :],
                                          in_=st2[0:w_b, :])
                    else:
                        nc.sync.dma_start(out=out2p[b * 128:b * 128 + w_b, :],
                                          in_=o[0:w_b, :])

            # ---- stage A, layer 1 (full table, replicated) + self slab ----
            SL = 16 * 128
            views1 = [(xTg[:, s0:min(s0 + SL, TROWS)], s0)
                      for s0 in range(0, TROWS, SL)]
            _stage_a(nc, (sbA, psA), views1, w1_sb, T1, IN)
            nc.sync.dma_start(out=T1[SENT_ROW:SENT_ROW + 1, :], in_=sent1_sb[:])
            viewsS = [(xTs[:, s0:min(s0 + SL, PERC)], s0)
                      for s0 in range(0, PERC, SL)]
            _stage_a(nc, (sbA, psA), viewsS, w1_sb, Ts1, IN)

            # ---- layer 1 edges ----
            edge_layer(T1, Ts1, asr1_sb, adr1_sb, b1r_sb, True)

            # ---- allgather o1T ----
            nc.gpsimd.collective_compute(
                "AllGather", mybir.AluOpType.bypass,
                replica_groups=[list(range(NCORES))],
                ins=[o1T[:].opt()], outs=[o1Tg[:].opt()])

            # ---- stage A, layer 2 (replicated from o1Tg) ----
            views2 = []
            for r in range(NCORES):
                for p0 in range(0, PERC, SL):
                    views2.append((o1Tg[r * F:(r + 1) * F, p0:min(p0 + SL, PERC)],
                                   r * SLAB + p0))
            _stage_a(nc, (sbA, psA), views2, w2_sb, T2, F)
            nc.sync.dma_start(out=T2[SENT_ROW:SENT_ROW + 1, :], in_=sent2_sb[:])

            # ---- layer 2 edges ----
            edge_layer(T2, Ts2, asr2_sb, adr2_sb, b2r_sb, False)

    nc.compile()
    return nc


_CACHE = {}


def _prepare(x, edge_index, W1, att_src1, att_dst1, b1, W2, att_src2,
             att_dst2, b2):
    x = np.asarray(x, np.float32)
    edge_index = np.asarray(edge_index, np.int64)
    key = hash(edge_index.tobytes())
    if key in _CACHE:
        meta, nc = _CACHE[key]
    else:
        meta = _preprocess(edge_index)
        nc = _build_program(meta)
        _CACHE[key] = (meta, nc)

    bf = ml_dtypes.bfloat16
    W1b = np.asarray(W1, np.float32).astype(bf)
    W2b = np.asarray(W2, np.float32).astype(bf)

    def att_rep(a):
        return np.broadcast_to(
            np.asarray(a, np.float32).reshape(H * F).astype(bf), (128, H * F)
        ).copy()

    def sent_row(a_src):
        Afull = np.zeros((H, H * F))
        a = np.asarray(a_src, np.float64)
        for h in range(H):
            Afull[h, h * F:(h + 1) * F] = a[h]
        v, *_ = np.linalg.lstsq(Afull, -300.0 * np.ones(H), rcond=None)
        return np.ascontiguousarray(v.astype(bf)).view(np.float32).reshape(1, RW)

    # x columns in g-order (junk cols zero)
    xb = x.astype(bf)
    arr = np.zeros((TROWS, IN), bf)
    arr[meta["row_of"]] = xb
    xTg = np.ascontiguousarray(arr.T)

    common = dict(
        xTg=xTg, w1=W1b, w2=W2b,
        asr1=att_rep(att_src1), adr1=att_rep(att_dst1),
        asr2=att_rep(att_src2), adr2=att_rep(att_dst2),
        identb=np.eye(128, dtype=bf), identf=np.eye(128, dtype=np.float32),
        sent1=sent_row(att_src1), sent2=sent_row(att_src2),
        b1r=np.broadcast_to(np.asarray(b1, np.float32), (128, F)).copy(),
        b2r=np.broadcast_to(np.asarray(b2, np.float32), (128, F)).copy(),
    )
    in_maps = []
    for c in range(NCORES):
        xTs = np.ascontiguousarray(xb[meta["perms"][c]].T)
        in_maps.append(dict(common, gidx=meta["gidx"][c], xTs=xTs))
    return nc, in_maps, meta


def _assemble(meta, results):
    out = np.empty((N, F), np.float32)
    for c in range(NCORES):
        out[meta["perms"][c]] = results[c]["out2p"]
    return out


def kernel(**inputs):
    nc, in_maps, meta = _prepare(**inputs)
    res = run_bass_kernel_spmd(nc, in_maps, core_ids=list(range(NCORES)))
    return _assemble(meta, res.results)


def run_traced(**inputs):
    """Profiled run; returns BassKernelResults (exec_time_ns etc.)."""
    nc, in_maps, meta = _prepare(**inputs)
    res = run_bass_kernel_spmd(nc, in_maps, core_ids=list(range(NCORES)),
                               trace=True)
    res.gat_output = _assemble(meta, res.results)
    return res


# revision 22
# speedup vs baseline: 1.7500x; 1.2130x over previous
"""2-layer GAT (PyG GATConv, concat=False, self-loops) on 8 Trainium2 cores.

Design (v2): nodes/edges partitioned by destination across 8 cores; each
core owns a 6250-dst range, dsts degree-sorted into 49 blocks of 128 PSUM
lanes.  Per-edge source features are fetched with dma_gather from a
256B-per-node table in HBM (pure bf16 h, h-major); per-edge attention
logits are recomputed on-chip with a DVE dot against a replicated a_src
row, so rows stay at the 256B dma_gather minimum.  The node table lives in
g-space (8 slabs of 6251 rows, one junk row per slab); one int16 index
stream serves both layers.  int16 gather indices cover 32767 rows, so
edges are split between two overlapping views (lo=[0,32767),
hi=[17241,50008)) with per-lane balancing of the 31% of sources that both
views can address.  Self-loop edges are excluded from the gather: each
core streams its own dst rows sequentially (also providing a_dst) and adds
the self contribution in the epilogue.  Padding slots point at a sentinel
row v with a_src@v = -300 so exp(LRelu(...)) ~ 0.  Scatter is free: slots
accumulate into PSUM via identity-weight matmuls (3 columns per matmul).
Layer boundary: AllGather of the bf16-transposed layer-1 output, then a
replicated stage-A rebuilds the layer-2 table on every core.
"""
import sys
sys.path.insert(0, "/opt/trn_rl_repo")

import numpy as np
import ml_dtypes

import concourse.bass as bass
import concourse.bacc as bacc
import concourse.mybir as mybir
from concourse.bass_utils import run_bass_kernel_spmd
from concourse.tile import TileContext

N = 50000
E = 1600000
IN = 128
H = 4
F = 32
NEG = 0.2
NCORES = 8
PERC = N // NCORES           # 6250
SLAB = PERC + 1              # 6251 (junk row at pos 6250)
TROWS = NCORES * SLAB        # 50008
NBLK = (PERC + 127) // 128   # 49
LO_END = 32767               # lo view = rows [0, 32767)
HI_START = TROWS - 32767     # 17241; hi view = rows [17241, 50008)
SENT_ROW = 3 * SLAB + PERC   # 25003 (core 3's junk row; inside the overlap)
RW = 64                      # table row: 64 f32 slots = 256B (128 bf16 h)
MAXC = int(__import__('os').environ.get('GAT_MAXC', '15'))  # max gather columns per dma_gather call
GRP = 3                      # aggregation matmul: 3 slot-columns per matmul


def _chunks(n):
    return [min(MAXC, n - s) for s in range(0, n, MAXC)]


def _pack_idx(idx_flat):
    """[n] -> [128, n/16] int16; idx i -> (partition i%16, col i//16), x8."""
    n = idx_flat.shape[0]
    assert n % 16 == 0
    a = idx_flat.reshape(n // 16, 16).T.astype(np.int16)
    return np.ascontiguousarray(np.tile(a, (8, 1)))


def _preprocess(edge_index):
    """Index preprocessing. Returns per-core gidx + shared structure."""
    src0 = edge_index[0].astype(np.int64)
    dst0 = edge_index[1].astype(np.int64)
    deg = np.bincount(dst0, minlength=N)          # gather-degree (no self)

    def build_perms(lo_cnt):
        perms = []
        pos_of = np.empty(N, np.int64)
        for c in range(NCORES):
            ids = np.arange(c * PERC, (c + 1) * PERC)
            order = np.lexsort((-lo_cnt[ids], -(deg[ids] // 4)))
            perm = ids[order]
            perms.append(perm)
            pos_of[perm] = np.arange(PERC)
        return perms, pos_of

    # pass 1: degree only -> provisional rows -> per-dst must-lo counts
    perms, pos_of = build_perms(np.zeros(N, np.int64))
    for _ in range(2):
        row_of = (np.arange(N) // PERC) * SLAB + pos_of
        must_lo = row_of[src0] < HI_START
        lo_cnt = np.bincount(dst0[must_lo], minlength=N)
        perms, pos_of = build_perms(lo_cnt)

    row_of = (np.arange(N) // PERC) * SLAB + pos_of
    r_src = row_of[src0]
    kind = np.where(r_src < HI_START, 0, np.where(r_src >= LO_END, 1, 2))

    lane_all = pos_of[dst0]
    core_all = dst0 // PERC
    blk_all = lane_all // 128

    # per-block slot counts nl/nh (cross-core maxima)
    A = np.zeros(NBLK, np.int64)
    B = np.zeros(NBLK, np.int64)
    C = np.zeros(NBLK, np.int64)
    cnt_l = np.zeros((NCORES, PERC), np.int64)
    cnt_h = np.zeros((NCORES, PERC), np.int64)
    cnt_f = np.zeros((NCORES, PERC), np.int64)
    for c in range(NCORES):
        m = core_all == c
        lane = lane_all[m]
        k = kind[m]
        cnt_l[c] = np.bincount(lane[k == 0], minlength=PERC)
        cnt_h[c] = np.bincount(lane[k == 1], minlength=PERC)
        cnt_f[c] = np.bincount(lane[k == 2], minlength=PERC)
    for b in range(NBLK):
        sl = slice(b * 128, min((b + 1) * 128, PERC))
        A[b] = cnt_l[:, sl].max()
        B[b] = cnt_h[:, sl].max()
        C[b] = (cnt_l[:, sl] + cnt_h[:, sl] + cnt_f[:, sl]).max()
    nl = A.copy()
    nh = np.maximum(B, C - A)
    S = int((nl + nh).sum())
    col_lo = np.zeros(NBLK, np.int64)   # column start of block's lo run
    col_hi = np.zeros(NBLK, np.int64)
    col = 0
    for b in range(NBLK):
        col_lo[b] = col
        col += nl[b]
        col_hi[b] = col
        col += nh[b]
    assert col == S

    # per-core column fill
    gidx = []
    sent_lo, sent_hi = SENT_ROW, SENT_ROW - HI_START
    for c in range(NCORES):
        m = core_all == c
        lane = lane_all[m]
        k = kind[m].copy()
        rows = r_src[m]
        # flex -> lo for the first a_i of each lane, rest -> hi
        nh_of_lane = nh[np.arange(PERC) // 128]
        a_i = np.maximum(0, cnt_h[c] + cnt_f[c] - nh_of_lane)
        # order edges by (lane, kind-rank, row); flex edges get rank by
        # position so the first a_i go to lo
        order = np.lexsort((rows, k, lane))
        lane_s, k_s, rows_s = lane[order], k[order], rows[order]
        # cumcount within (lane, kind)
        start = np.r_[True, (lane_s[1:] != lane_s[:-1]) | (k_s[1:] != k_s[:-1])]
        grp_id = np.cumsum(start) - 1
        first_pos = np.full(grp_id[-1] + 1, 1 << 62, np.int64)
        np.minimum.at(first_pos, grp_id, np.arange(len(grp_id)))
        cum = np.arange(len(grp_id)) - first_pos[grp_id]
        # stream + slot per edge
        is_lo = (k_s == 0) | ((k_s == 2) & (cum < a_i[lane_s]))
        slot = np.where(
            k_s == 0, cum,                                   # must-lo
            np.where(k_s == 1, cum,                          # must-hi
                     np.where(is_lo, cnt_l[c][lane_s] + cum,  # flex->lo
                              cnt_h[c][lane_s] + cum - a_i[lane_s])))  # flex->hi
        b_s = lane_s // 128
        colidx = np.where(is_lo, col_lo[b_s] + slot, col_hi[b_s] + slot)
        grid = np.full((S, 128), -1, np.int64)
        grid[colidx, lane_s % 128] = rows_s
        # sentinels + view-relative index
        lo_cols = np.zeros(S, bool)
        for b in range(NBLK):
            lo_cols[col_lo[b]:col_lo[b] + nl[b]] = True
        grid[lo_cols] = np.where(grid[lo_cols] < 0, sent_lo, grid[lo_cols])
        grid[~lo_cols] = np.where(grid[~lo_cols] < 0, sent_hi + HI_START,
                                  grid[~lo_cols]) - HI_START
        assert grid.min() >= 0 and grid.max() < LO_END
        gidx.append(_pack_idx(grid.reshape(-1)))

    return dict(gidx=gidx, nl=nl, nh=nh, S=S, col_lo=col_lo, col_hi=col_hi,
                perms=perms, pos_of=pos_of, row_of=row_of)


def _stage_a(nc, pools, views, w_sb, tbl, kdim):
    """h = lhsT.T @ W -> bf16 rows packed into tbl.

    views: list of (lhsT_view [kdim, W], row0); W <= 16*128. Rows go out as
    slab DMAs to tbl[row0:row0+W] (f32 [*, 64] holding 128 bf16).
    """
    sb, ps = pools
    f32, bf16 = mybir.dt.float32, mybir.dt.bfloat16
    for (view, row0) in views:
        Wt = view.shape[1]
        nt = (Wt + 127) // 128
        xsb = sb.tile([kdim, 16 * 128], bf16, tag="xa")
        nc.sync.dma_start(out=xsb[:, 0:Wt], in_=view)
        stg = sb.tile([128, 16, RW], f32, tag="sa")
        for t in range(nt):
            w = min(128, Wt - t * 128)
            psum = ps.tile([128, 128], f32, tag="pa")
            nc.tensor.matmul(out=psum[0:w, :], lhsT=xsb[:, t * 128:t * 128 + w],
                             rhs=w_sb[:], start=True, stop=True)
            nc.vector.tensor_copy(
                out=stg[0:w, t, :].bitcast(bf16),
                in_=psum[0:w, :])
        nfull = Wt // 128
        if nfull:
            nc.sync.dma_start(
                out=tbl[row0:row0 + nfull * 128, :].rearrange(
                    "(t p) c -> p t c", p=128),
                in_=stg[:, 0:nfull, :])
        if Wt % 128:
            nc.sync.dma_start(out=tbl[row0 + nfull * 128:row0 + Wt, :],
                              in_=stg[0:Wt % 128, nfull, :])


def _build_program(meta):
    nl, nh, S = meta["nl"], meta["nh"], meta["S"]
    nc = bacc.Bacc("TRN2", target_bir_lowering=False, debug=False,
                   num_devices=NCORES, dynamic_dma_scratch_size=int(__import__('os').environ.get('GAT_SCRATCH', '65536')))

    f32, bf16, i16 = mybir.dt.float32, mybir.dt.bfloat16, mybir.dt.int16
    xTg = nc.dram_tensor("xTg", [IN, TROWS], bf16, kind="ExternalInput")
    xTs = nc.dram_tensor("xTs", [IN, PERC], bf16, kind="ExternalInput")
    w1 = nc.dram_tensor("w1", [IN, H * F], bf16, kind="ExternalInput")
    w2 = nc.dram_tensor("w2", [F, H * F], bf16, kind="ExternalInput")
    asr1 = nc.dram_tensor("asr1", [128, H * F], bf16, kind="ExternalInput")
    adr1 = nc.dram_tensor("adr1", [128, H * F], bf16, kind="ExternalInput")
    asr2 = nc.dram_tensor("asr2", [128, H * F], bf16, kind="ExternalInput")
    adr2 = nc.dram_tensor("adr2", [128, H * F], bf16, kind="ExternalInput")
    gidx = nc.dram_tensor("gidx", [128, S * 8], i16, kind="ExternalInput")
    identb = nc.dram_tensor("identb", [128, 128], bf16, kind="ExternalInput")
    identf = nc.dram_tensor("identf", [128, 128], f32, kind="ExternalInput")
    sent1 = nc.dram_tensor("sent1", [1, RW], f32, kind="ExternalInput")
    sent2 = nc.dram_tensor("sent2", [1, RW], f32, kind="ExternalInput")
    b1r = nc.dram_tensor("b1r", [128, F], f32, kind="ExternalInput")
    b2r = nc.dram_tensor("b2r", [128, F], f32, kind="ExternalInput")

    T1 = nc.dram_tensor("T1", [TROWS, RW], f32)
    T2 = nc.dram_tensor("T2", [TROWS, RW], f32)
    Ts1 = nc.dram_tensor("Ts1", [PERC, RW], f32)
    Ts2 = nc.dram_tensor("Ts2", [PERC, RW], f32)
    o1T = nc.dram_tensor("o1T", [F, PERC], bf16)
    o1Tg = nc.dram_tensor("o1Tg", [NCORES * F, PERC], bf16, addr_space="Shared")
    out2p = nc.dram_tensor("out2p", [PERC, F], f32, kind="ExternalOutput")

    NC3 = ((MAXC + GRP - 1) // GRP) * GRP  # rhs tile columns (33)

    with TileContext(nc) as tc:
        with (
            tc.tile_pool(name="cons", bufs=1) as cons,
            tc.tile_pool(name="sbA", bufs=3) as sbA,
            tc.tile_pool(name="psA", bufs=2, space="PSUM") as psA,
            tc.tile_pool(name="dp", bufs=2) as dp,
            tc.tile_pool(name="gp", bufs=4) as gp,
            tc.tile_pool(name="tp", bufs=3) as tp,
            tc.tile_pool(name="rp", bufs=3) as rp,
            tc.tile_pool(name="ep", bufs=8) as ep,
            tc.tile_pool(name="pp", bufs=1) as pp,
            tc.tile_pool(name="psE", bufs=3, space="PSUM") as psE,
            tc.tile_pool(name="psT", bufs=1, space="PSUM") as psT,
        ):
            identb_sb = cons.tile([128, 128], bf16)
            nc.sync.dma_start(out=identb_sb[:], in_=identb[:, :])
            identf_sb = cons.tile([128, 128], f32)
            nc.sync.dma_start(out=identf_sb[:], in_=identf[:, :])
            w1_sb = cons.tile([IN, H * F], bf16)
            nc.sync.dma_start(out=w1_sb[:], in_=w1[:, :])
            w2_sb = cons.tile([F, H * F], bf16)
            nc.sync.dma_start(out=w2_sb[:], in_=w2[:, :])
            asr1_sb = cons.tile([128, H * F], bf16)
            nc.sync.dma_start(out=asr1_sb[:], in_=asr1[:, :])
            adr1_sb = cons.tile([128, H * F], bf16)
            nc.sync.dma_start(out=adr1_sb[:], in_=adr1[:, :])
            asr2_sb = cons.tile([128, H * F], bf16)
            nc.sync.dma_start(out=asr2_sb[:], in_=asr2[:, :])
            adr2_sb = cons.tile([128, H * F], bf16)
            nc.sync.dma_start(out=adr2_sb[:], in_=adr2[:, :])
            b1r_sb = cons.tile([128, F], f32)
            nc.sync.dma_start(out=b1r_sb[:], in_=b1r[:, :])
            b2r_sb = cons.tile([128, F], f32)
            nc.sync.dma_start(out=b2r_sb[:], in_=b2r[:, :])
            sent1_sb = cons.tile([1, RW], f32)
            nc.sync.dma_start(out=sent1_sb[:], in_=sent1[:, :])
            sent2_sb = cons.tile([1, RW], f32)
            nc.sync.dma_start(out=sent2_sb[:], in_=sent2[:, :])
            gidx_sb = cons.tile([128, S * 8], i16)
            nc.sync.dma_start(out=gidx_sb[:], in_=gidx[:, :])

            def d_phase(tself, asr_sb, adr_sb):
                """Per-block dst-slab read + dot products; returns
                pre[b] = (adl, rhs_s) tiles that persist until the g-phase."""
                pre = {}
                for b in range(NBLK):
                    w_b = min(128, PERC - b * 128)
                    dt = dp.tile([128, RW], f32, tag="dt")
                    if w_b < 128:
                        nc.vector.memset(dt[:], 0.0)
                    nc.sync.dma_start(out=dt[0:w_b, :],
                                      in_=tself[b * 128:b * 128 + w_b, :])
                    dhb = dt[:].bitcast(bf16)          # [128, 128] h-major
                    tmpd = tp.tile([128, 128], bf16, tag="tmpd")
                    nc.vector.tensor_tensor(out=tmpd[:], in0=dhb, in1=asr_sb[:],
                                            op=mybir.AluOpType.mult)
                    asl = ep.tile([128, H], f32, tag="asl")
                    nc.vector.tensor_reduce(
                        out=asl[:], in_=tmpd[:].rearrange("p (h f) -> p h f", f=F),
                        axis=mybir.AxisListType.X, op=mybir.AluOpType.add)
                    tmpd2 = tp.tile([128, 128], bf16, tag="tmpd2")
                    nc.vector.tensor_tensor(out=tmpd2[:], in0=dhb, in1=adr_sb[:],
                                            op=mybir.AluOpType.mult)
                    adl = pp.tile([128, H], f32, tag=f"adl{b}")
                    nc.vector.tensor_reduce(
                        out=adl[:], in_=tmpd2[:].rearrange("p (h f) -> p h f", f=F),
                        axis=mybir.AxisListType.X, op=mybir.AluOpType.add)
                    # self edge: e = LRelu(asl + adl); w = exp(e)
                    es = ep.tile([128, H], f32, tag="es")
                    nc.vector.tensor_tensor(out=es[:], in0=asl[:], in1=adl[:],
                                            op=mybir.AluOpType.add)
                    es2 = ep.tile([128, H], f32, tag="es2")
                    nc.vector.tensor_scalar(out=es2[:], in0=es[:], scalar1=NEG,
                                            scalar2=None, op0=mybir.AluOpType.mult)
                    nc.vector.tensor_tensor(out=es2[:], in0=es2[:], in1=es[:],
                                            op=mybir.AluOpType.max)
                    wsb = ep.tile([128, H], bf16, tag="wsb")
                    nc.scalar.activation(out=wsb[:], in_=es2[:],
                                         func=mybir.ActivationFunctionType.Exp)
                    rhs_s = pp.tile([128, 4 + H * F], f32, tag=f"rs{b}")
                    nc.vector.tensor_copy(out=rhs_s[:, 0:4], in_=wsb[:])
                    nc.vector.tensor_tensor(
                        out=rhs_s[:, 4:132].rearrange("p (h f) -> p h f", f=F),
                        in0=dhb.rearrange("p (h f) -> p h f", f=F),
                        in1=wsb[:].unsqueeze(2).to_broadcast([128, H, F]),
                        op=mybir.AluOpType.mult)
                    pre[b] = (adl, rhs_s)
                return pre

            def edge_layer(tbl, pre, asr_sb, bias_sb, is_layer1):
                tbl_lo = tbl[0:LO_END, :]
                tbl_hi = tbl[HI_START:TROWS, :]
                for b in range(NBLK):
                    w_b = min(128, PERC - b * 128)
                    adl, rhs_s = pre[b]

                    # ---- gathered slots ----
                    psum = psE.tile([128, GRP * 132], f32, tag="acc")
                    n_tri = sum((cc + GRP - 1) // GRP
                                for nn in (int(nl[b]), int(nh[b]))
                                for cc in _chunks(nn))
                    tri = 0
                    for half in range(2):
                        ncols_all = int(nl[b]) if half == 0 else int(nh[b])
                        col0 = int(meta["col_lo"][b]) if half == 0 \
                            else int(meta["col_hi"][b])
                        view = tbl_lo if half == 0 else tbl_hi
                        for s0 in range(0, ncols_all, MAXC):
                            ncc = min(MAXC, ncols_all - s0)
                            nc3 = ((ncc + GRP - 1) // GRP) * GRP
                            g = gp.tile([128, MAXC, RW], f32, tag="g")
                            nc.gpsimd.dma_gather(
                                g[:, 0:ncc, :], view,
                                gidx_sb[:, (col0 + s0) * 8:(col0 + s0 + ncc) * 8],
                                ncc * 128, ncc * 128, RW,
                                single_packet=(ncc * 128 <= 1008))
                            gb = g[:].bitcast(bf16)    # [128, MAXC, 128]
                            tmp = tp.tile([128, MAXC, 128], bf16, tag="tmp")
                            nc.vector.tensor_tensor(
                                out=tmp[:, 0:ncc, :], in0=gb[:, 0:ncc, :],
                                in1=asr_sb[:].unsqueeze(1).to_broadcast(
                                    [128, ncc, H * F]),
                                op=mybir.AluOpType.mult)
                            al = ep.tile([128, MAXC, H], f32, tag="al")
                            nc.vector.tensor_reduce(
                                out=al[:, 0:ncc, :],
                                in_=tmp[:, 0:ncc, :].rearrange(
                                    "p n (h f) -> p n h f", f=F),
                                axis=mybir.AxisListType.X,
                                op=mybir.AluOpType.add)
                            nc.vector.tensor_tensor(
                                out=al[:, 0:ncc, :], in0=al[:, 0:ncc, :],
                                in1=adl[:].unsqueeze(1).to_broadcast(
                                    [128, ncc, H]),
                                op=mybir.AluOpType.add)
                            alf = al[:, 0:ncc, :].rearrange("p n h -> p (n h)")
                            e2 = ep.tile([128, MAXC, H], f32, tag="e2")
                            e2f = e2[:, 0:ncc, :].rearrange("p n h -> p (n h)")
                            nc.vector.tensor_scalar(
                                out=e2f, in0=alf, scalar1=NEG, scalar2=None,
                                op0=mybir.AluOpType.mult)
                            nc.vector.tensor_tensor(
                                out=e2f, in0=e2f, in1=alf,
                                op=mybir.AluOpType.max)
                            rhs = rp.tile([128, NC3, 132], bf16, tag="rhs")
                            if nc3 > ncc:
                                nc.vector.memset(rhs[:, ncc:nc3, :], 0.0)
                            nc.scalar.activation(
                                out=rhs[:, 0:ncc, 0:4], in_=e2[:, 0:ncc, :],
                                func=mybir.ActivationFunctionType.Exp)
                            nc.vector.tensor_tensor(
                                out=rhs[:, 0:ncc, 4:132].rearrange(
                                    "p n (h f) -> p n h f", f=F),
                                in0=gb[:, 0:ncc, :].rearrange(
                                    "p n (h f) -> p n h f", f=F),
                                in1=rhs[:, 0:ncc, 0:4].unsqueeze(3).to_broadcast(
                                    [128, ncc, H, F]),
                                op=mybir.AluOpType.mult)
                            for t in range(nc3 // GRP):
                                nc.tensor.matmul(
                                    out=psum[:],
                                    lhsT=identb_sb[:],
                                    rhs=rhs[:, t * GRP:(t + 1) * GRP, :].rearrange(
                                        "p a b -> p (a b)"),
                                    start=(tri == 0), stop=(tri == n_tri - 1))
                                tri += 1
                    assert tri == n_tri

                    # ---- epilogue ----
                    U = ep.tile([128, 132], f32, tag="U")
                    nc.vector.tensor_tensor(out=U[:], in0=rhs_s[:],
                                            in1=psum[:, 0:132],
                                            op=mybir.AluOpType.add)
                    nc.vector.tensor_tensor(out=U[:], in0=U[:],
                                            in1=psum[:, 132:264],
                                            op=mybir.AluOpType.add)
                    nc.vector.tensor_tensor(out=U[:], in0=U[:],
                                            in1=psum[:, 264:396],
                                            op=mybir.AluOpType.add)
                    sden = ep.tile([128, H], f32, tag="sden")
                    nc.vector.tensor_scalar(out=sden[:], in0=U[:, 0:4],
                                            scalar1=1e-16, scalar2=None,
                                            op0=mybir.AluOpType.add)
                    rv = ep.tile([128, H], f32, tag="rv")
                    nc.vector.reciprocal(out=rv[:], in_=sden[:])
                    nc.vector.tensor_scalar(out=rv[:], in0=rv[:], scalar1=1.0 / H,
                                            scalar2=None,
                                            op0=mybir.AluOpType.mult)
                    m = ep.tile([128, H * F], f32, tag="m")
                    nc.vector.tensor_tensor(
                        out=m[:].rearrange("p (h f) -> p h f", f=F),
                        in0=U[:, 4:132].rearrange("p (h f) -> p h f", f=F),
                        in1=rv[:].unsqueeze(2).to_broadcast([128, H, F]),
                        op=mybir.AluOpType.mult)
                    o = ep.tile([128, F], f32, tag="o")
                    nc.vector.tensor_tensor(out=o[:], in0=m[:, 0:F],
                                            in1=m[:, F:2 * F],
                                            op=mybir.AluOpType.add)
                    o2 = ep.tile([128, F], f32, tag="o2t")
                    nc.vector.tensor_tensor(out=o2[:], in0=m[:, 2 * F:3 * F],
                                            in1=m[:, 3 * F:4 * F],
                                            op=mybir.AluOpType.add)
                    nc.vector.tensor_tensor(out=o[:], in0=o[:], in1=o2[:],
                                            op=mybir.AluOpType.add)
                    nc.vector.tensor_tensor(out=o[:], in0=o[:], in1=bias_sb[:],
                                            op=mybir.AluOpType.add)
                    if is_layer1:
                        # ELU
                        m0 = ep.tile([128, F], f32, tag="m0")
                        nc.vector.tensor_scalar(out=m0[:], in0=o[:], scalar1=0.0,
                                                scalar2=None,
                                                op0=mybir.AluOpType.min)
                        em = ep.tile([128, F], f32, tag="em")
                        nc.scalar.activation(out=em[:], in_=m0[:],
                                             func=mybir.ActivationFunctionType.Exp)
                        nc.vector.tensor_scalar(out=em[:], in0=em[:], scalar1=-1.0,
                                                scalar2=None,
                                                op0=mybir.AluOpType.add)
                        nc.vector.tensor_tensor(out=o[:], in0=o[:], in1=em[:],
                                                op=mybir.AluOpType.max)
                        # transpose -> o1T (bf16) + h2 = o1 @ W2 -> Ts2
                        pT = psT.tile([F, 128], f32, tag="pT")
                        nc.tensor.transpose(out=pT[:], in_=o[:],
                                            identity=identf_sb[:])
                        oT = ep.tile([F, 128], bf16, tag="oT")
                        nc.vector.tensor_copy(out=oT[:], in_=pT[:])
                        nc.sync.dma_start(out=o1T[:, b * 128:b * 128 + w_b],
                                          in_=oT[:, 0:w_b])
                        ps2 = psT.tile([128, H * F], f32, tag="ps2")
                        nc.tensor.matmul(out=ps2[:], lhsT=oT[:], rhs=w2_sb[:],
                                         start=True, stop=True)
                        st2 = ep.tile([128, RW], f32, tag="st2")
                        nc.vector.tensor_copy(out=st2[:].bitcast(bf16),
                                              in_=ps2[:])
                        nc.sync.dma_start(out=Ts2[b * 128:b * 128 + w_b, :],
                                          in_=st2[0:w_b, :])
                    else:
                        nc.sync.dma_start(out=out2p[b * 128:b * 128 + w_b, :],
                                          in_=o[0:w_b, :])

            # ---- stage A, layer 1: self slab first, then full table ----
            SL = 16 * 128
            viewsS = [(xTs[:, s0:min(s0 + SL, PERC)], s0)
                      for s0 in range(0, PERC, SL)]
            _stage_a(nc, (sbA, psA), viewsS, w1_sb, Ts1, IN)
            views1 = [(xTg[:, s0:min(s0 + SL, TROWS)], s0)
                      for s0 in range(0, TROWS, SL)]
            _stage_a(nc, (sbA, psA), views1, w1_sb, T1, IN)
            nc.sync.dma_start(out=T1[SENT_ROW:SENT_ROW + 1, :], in_=sent1_sb[:])

            # ---- layer 1 edges (D-phase overlaps stage-A1's table build) ----
            pre1 = d_phase(Ts1, asr1_sb, adr1_sb)
            edge_layer(T1, pre1, asr1_sb, b1r_sb, True)

            # ---- allgather o1T; layer-2 D-phase fills the window ----
            nc.gpsimd.collective_compute(
                "AllGather", mybir.AluOpType.bypass,
                replica_groups=[list(range(NCORES))],
                ins=[o1T[:].opt()], outs=[o1Tg[:].opt()])
            pre2 = d_phase(Ts2, asr2_sb, adr2_sb)

            # ---- stage A, layer 2 (replicated from o1Tg) ----
            views2 = []
            for r in range(NCORES):
                for p0 in range(0, PERC, SL):
                    views2.append((o1Tg[r * F:(r + 1) * F, p0:min(p0 + SL, PERC)],
                                   r * SLAB + p0))
            _stage_a(nc, (sbA, psA), views2, w2_sb, T2, F)
            nc.sync.dma_start(out=T2[SENT_ROW:SENT_ROW + 1, :], in_=sent2_sb[:])

            # ---- layer 2 edges ----
            edge_layer(T2, pre2, asr2_sb, b2r_sb, False)

    nc.compile()
    return nc


_CACHE = {}


def _prepare(x, edge_index, W1, att_src1, att_dst1, b1, W2, att_src2,
             att_dst2, b2):
    x = np.asarray(x, np.float32)
    edge_index = np.asarray(edge_index, np.int64)
    key = hash(edge_index.tobytes())
    if key in _CACHE:
        meta, nc = _CACHE[key]
    else:
        meta = _preprocess(edge_index)
        nc = _build_program(meta)
        _CACHE[key] = (meta, nc)

    bf = ml_dtypes.bfloat16
    W1b = np.asarray(W1, np.float32).astype(bf)
    W2b = np.asarray(W2, np.float32).astype(bf)

    def att_rep(a):
        return np.broadcast_to(
            np.asarray(a, np.float32).reshape(H * F).astype(bf), (128, H * F)
        ).copy()

    def sent_row(a_src):
        Afull = np.zeros((H, H * F))
        a = np.asarray(a_src, np.float64)
        for h in range(H):
            Afull[h, h * F:(h + 1) * F] = a[h]
        v, *_ = np.linalg.lstsq(Afull, -300.0 * np.ones(H), rcond=None)
        return np.ascontiguousarray(v.astype(bf)).view(np.float32).reshape(1, RW)

    # x columns in g-order (junk cols zero)
    xb = x.astype(bf)
    arr = np.zeros((TROWS, IN), bf)
    arr[meta["row_of"]] = xb
    xTg = np.ascontiguousarray(arr.T)

    common = dict(
        xTg=xTg, w1=W1b, w2=W2b,
        asr1=att_rep(att_src1), adr1=att_rep(att_dst1),
        asr2=att_rep(att_src2), adr2=att_rep(att_dst2),
        identb=np.eye(128, dtype=bf), identf=np.eye(128, dtype=np.float32),
        sent1=sent_row(att_src1), sent2=sent_row(att_src2),
        b1r=np.broadcast_to(np.asarray(b1, np.float32), (128, F)).copy(),
        b2r=np.broadcast_to(np.asarray(b2, np.float32), (128, F)).copy(),
    )
    in_maps = []
    for c in range(NCORES):
        xTs = np.ascontiguousarray(xb[meta["perms"][c]].T)
        in_maps.append(dict(common, gidx=meta["gidx"][c], xTs=xTs))
    return nc, in_maps, meta


def _assemble(meta, results):
    out = np.empty((N, F), np.float32)
    for c in range(NCORES):
        out[meta["perms"][c]] = results[c]["out2p"]
    return out


def kernel(**inputs):
    nc, in_maps, meta = _prepare(**inputs)
    res = run_bass_kernel_spmd(nc, in_maps, core_ids=list(range(NCORES)))
    return _assemble(meta, res.results)


def run_traced(**inputs):
    """Profiled run; returns BassKernelResults (exec_time_ns etc.)."""
    nc, in_maps, meta = _prepare(**inputs)
    res = run_bass_kernel_spmd(nc, in_maps, core_ids=list(range(NCORES)),
                               trace=True)
    res.gat_output = _assemble(meta, res.results)
    return res
